# revision 1
# baseline (speedup 1.0000x reference)
"""Adaptive self-attention block on 8 trn2 NeuronCores.

Sharding: data-parallel over batch B=8 (one image per core); the small
weights are replicated to every core. All shapes hardcoded from the
problem spec: B=8, H=W=128, C=192, NH=6, SPLIT=(8,16).
"""
import numpy as np
import jax
import jax.numpy as jnp

B0, H0, W0, C0, NH0 = 8, 128, 128, 192, 6
SPLIT = (8, 16)
BN_EPS = 1e-5
L0 = H0 * W0

_WEIGHT_NAMES = [
    "w_qkv", "b_qkv", "w_dw1", "b_dw1", "g_bn1", "be_bn1",
    "w_si1", "b_si1", "g_si", "be_si", "w_si2", "b_si2",
    "w_ci1", "b_ci1", "g_ci", "be_ci", "w_ci2", "b_ci2",
    "w_dw2", "b_dw2", "g_bn2", "be_bn2", "w_proj", "b_proj",
]


def _bn(x, g, b):
    return x * (g / jnp.sqrt(1.0 + BN_EPS))[:, None, None] + b[:, None, None]


def _gelu(x):
    return jax.nn.gelu(x, approximate=False)


def _dwconv3x3(x, w, b):
    # x:(C,H,W), w:(C,1,3,3) depthwise -> 9 shifted multiply-accumulates
    C, H, W = x.shape
    xp = jnp.pad(x, ((0, 0), (1, 1), (1, 1)))
    y = jnp.zeros_like(x)
    for dh in range(3):
        for dw in range(3):
            y = y + xp[:, dh:dh + H, dw:dw + W] * w[:, 0, dh, dw][:, None, None]
    return y + b[:, None, None]


def _win_attn(q, k, v, H, W, Hs, Ws, nh):
    L, Cb = q.shape
    hd = Cb // nh
    scale = hd ** (-0.5)
    T = Hs * Ws

    def to_win(t):  # (L,Cb) -> (nW, nh, T, hd)
        t = t.reshape(H // Hs, Hs, W // Ws, Ws, Cb)
        t = t.transpose(0, 2, 1, 3, 4).reshape(-1, T, nh, hd)
        return t.transpose(0, 2, 1, 3)

    qw, kw, vw = to_win(q), to_win(k), to_win(v)
    attn = jnp.einsum('wntd,wnsd->wnts', qw, kw,
                      preferred_element_type=jnp.float32) * (3.0 * scale)
    attn = jax.nn.softmax(attn, axis=-1).astype(q.dtype)
    out = jnp.einsum('wnts,wnsd->wntd', attn, 3.0 * vw)
    out = out.transpose(0, 2, 1, 3).reshape(-1, T, Cb)
    out = out.reshape(H // Hs, W // Ws, Hs, Ws, Cb)
    out = out.transpose(0, 2, 1, 3, 4).reshape(H, W, Cb)
    return out


def _forward_one(x, w_qkv, b_qkv, w_dw1, b_dw1, g_bn1, be_bn1,
                 w_si1, b_si1, g_si, be_si, w_si2, b_si2,
                 w_ci1, b_ci1, g_ci, be_ci, w_ci2, b_ci2,
                 w_dw2, b_dw2, g_bn2, be_bn2, w_proj, b_proj):
    # x: (L, C) one image; bulk math in bf16, output f32
    bf = jnp.bfloat16
    x = x.astype(bf)
    w_qkv, b_qkv, w_proj, b_proj = (t.astype(bf) for t in (w_qkv, b_qkv, w_proj, b_proj))
    w_dw1, b_dw1, w_dw2, b_dw2 = (t.astype(bf) for t in (w_dw1, b_dw1, w_dw2, b_dw2))
    g_bn1, be_bn1, g_bn2, be_bn2 = (t.astype(bf) for t in (g_bn1, be_bn1, g_bn2, be_bn2))
    w_si1, b_si1, g_si, be_si, w_si2, b_si2 = (
        t.astype(bf) for t in (w_si1, b_si1, g_si, be_si, w_si2, b_si2))
    w_ci1, b_ci1, g_ci, be_ci, w_ci2, b_ci2 = (
        t.astype(bf) for t in (w_ci1, b_ci1, g_ci, be_ci, w_ci2, b_ci2))
    H, W, C, L = H0, W0, C0, L0
    Ch = C // 2
    nh = NH0 // 2

    qkv = x @ w_qkv + b_qkv                      # (L, 3C)
    q = qkv[:, :C]
    k = qkv[:, C:2 * C]
    v = qkv[:, 2 * C:]

    fu1 = _win_attn(q[:, :Ch], k[:, :Ch], v[:, :Ch], H, W, SPLIT[0], SPLIT[1], nh)
    fu2 = _win_attn(q[:, Ch:], k[:, Ch:], v[:, Ch:], H, W, SPLIT[1], SPLIT[0], nh)
    fu = jnp.concatenate([fu1.reshape(L, Ch), fu2.reshape(L, Ch)], axis=-1)

    # raw reshape (not transpose), replicated from the source
    x_img = x.reshape(C, H, W)
    conv_x1 = _gelu(_bn(_dwconv3x3(x_img, w_dw1, b_dw1), g_bn1, be_bn1))

    cf = conv_x1.reshape(C, L)                   # (C, L)
    s = (w_si1 @ cf) + b_si1[:, None]            # (C8, L)
    s = _gelu(s.reshape(-1, H, W) * (g_si / jnp.sqrt(1.0 + BN_EPS))[:, None, None]
              + be_si[:, None, None]).reshape(-1, L)
    s = (w_si2 @ s) + b_si2[:, None]             # (1, L)
    spatial_map = s.reshape(L, 1)

    c = cf.astype(jnp.float32).mean(axis=1).astype(bf)   # (C,)
    c = w_ci1 @ c + b_ci1                        # (C8,)
    c = _gelu(c * (g_ci / jnp.sqrt(1.0 + BN_EPS)) + be_ci)
    c = w_ci2 @ c + b_ci2                        # (C,)
    channel_map = c.reshape(1, C)

    fu = fu + fu * jax.nn.sigmoid(spatial_map)
    fu_img = fu.T.reshape(C, H, W)
    fu_img = _gelu(_bn(_dwconv3x3(fu_img, w_dw2, b_dw2), g_bn2, be_bn2))
    fu = fu_img.reshape(C, L).T
    fu = fu + fu * jax.nn.sigmoid(channel_map)
    out = jnp.dot(fu, w_proj, preferred_element_type=jnp.float32)
    return (out + b_proj.astype(jnp.float32)).astype(bf)


_pmapped = None


def _get_pmapped():
    global _pmapped
    if _pmapped is None:
        _pmapped = jax.pmap(
            _forward_one,
            in_axes=(0,) + (None,) * len(_WEIGHT_NAMES),
            devices=jax.devices()[:8],
        )
    return _pmapped


def kernel(**inputs) -> np.ndarray:
    import ml_dtypes  # ships with jax; bf16 numpy dtype
    x = np.asarray(inputs["x"], dtype=np.float32).astype(ml_dtypes.bfloat16)
    ws = [np.asarray(inputs[n], dtype=np.float32) for n in _WEIGHT_NAMES]
    fn = _get_pmapped()
    out = fn(x, *ws)                                       # (8, L, C) bf16
    return np.asarray(jax.device_get(out)).astype(np.float32)

